# revision 48
# baseline (speedup 1.0000x reference)
"""Multi-head attention (B=2, N=2048, D=1024, H=16) on 8 Trainium2 cores.

Sharding: data-parallel over batch (2) x tensor-parallel over head groups (4).
Core c handles batch c//4, heads 4*(c%4) .. 4*(c%4)+3 (256 channels).

Per-core kernel (token-major PV + split-exp + fp8-residual variant):
  - x and the qkv weights ship as fp8e4m3 value+residual pairs; each
    projection runs three DoubleRow passes (x8*w8 + x8*wr8 + xr8*w8) at
    0.5 PE cycles/row with 256-deep contraction per instruction -- 25%
    fewer PE cycles than bf16 with ~0.13% per-element error. The weights
    are pre-scaled x32 on the host so their residuals clear the e4m3
    subnormal floor (without this the residual quantizes to ~2e-3 steps
    and costs 1.8%/element); the exact /32 rides the scale+bias drains.
    W_o / kT / qT stay bf16 (1 PE cycle/row at any width)
  - kT/qT stored [64, head, N] so every S matmul reads partition offset 0
    (offset-64 operands + sub-bank psum outputs misbehave on hw)
  - PV runs token-major: out[q-tokens, 1+64] with P as the stationary
    operand and [ones | v+v_bias] as the 65-wide moving operand, so each
    PV matmul costs 65 rows instead of 256 and the softmax denominator
    lands in psum column 0 per q-PARTITION; v_bias rides v65 exactly
    (sum p*(v+vb)/den = PV/den + vb)
  - normalize is then reciprocal + per-partition tensor_scalar multiplies
    (no gpsimd broadcast); o flips to channel-major for the O-projection
    via PE transposes (128 rows each) against an on-chip identity
  - exp is split per head: heads 0,1 true exp on ACT; heads 2,3 exp via
    one DVE op writing int16 bits interpreted as bf16 (Schraudolph:
    i = 128*log2(e)*scale*s + 16196.5, exact to ~3%; max rel err measured
    1.2e-2 vs 2e-2 budget). gpsimd cannot read PSUM on this hw, so only
    ACT+DVE carry psum work; the st tile is split [128,2,QW]+[128,2,QW]
    (sta->ACT, stb->DVE) so each exp engine recycles its own bank and the
    pt tiles are separate so the writers never serialize
  - query tiles of 256, key tiles of 128; per body: 4 S matmuls, 1 ACT exp,
    1 DVE schbf, 8 deferred 65-wide PV matmuls
  - PSUM (8 banks): 2x2 single-bank double-buffered sta/stb, 2 one-bank
    q-token accumulators (opened by one 260-wide zero matmul each),
    2 streamed chains
  - k-projection runs kt-round-wise while the per-kt-interleaved wk/x DMAs
    land; wq is prioritized right after x so q0 never waits; a cheap PE
    warmup chain keeps the p-state ramp hot
  - the in-order engines + counting semaphores couple any stalled PE
    instruction to every later exp: accumulator opens are deferred to
    body 2, PV groups drain one per body with the body-15 flush keeping
    the freshest group (it + the normalize ride body 0 of the next qt),
    transposes ride bodies 2-5 and o-projection pieces stream from body 10
Host: out[b] = sum of the 4 group partials + b_o.
"""

import sys

sys.path.insert(0, "/opt/trn_rl_repo")

import numpy as np

B, N, D, H = 2, 2048, 1024, 16
SUB = D // H  # 64
GROUPS = 4  # tensor-parallel head groups
NH = H // GROUPS  # 4 local heads per core
CH = NH * SUB  # 256 local channels
NCORES = 8
QW = 256  # query tile width


def build_nc(name="mha3", stage=40, ph=1, oat=7, pat=8, catchup=False, s_after=False, t_eng="dve", n_eng="dve", stg_eng="act"):
    from collections import deque

    import concourse.mybir as mybir
    from concourse import bacc
    from concourse import masks
    from concourse.tile import TileContext

    f32 = mybir.dt.float32
    f32r = mybir.dt.float32r
    bf16 = mybir.dt.bfloat16
    i16 = mybir.dt.int16
    Exp = mybir.ActivationFunctionType.Exp
    Ident = mybir.ActivationFunctionType.Identity
    mult = mybir.AluOpType.mult

    NT, DK, DO = N, D, D
    nh, ch = NH, CH
    KT = DK // 128  # 8 contraction ptiles
    CHT = ch // 128  # 2 channel ptiles
    TOKT = NT // 128  # 16 key ptiles
    QT = NT // QW  # 8 query tiles
    scale = SUB**-0.5

    f8 = mybir.dt.float8e4

    nc = bacc.Bacc(None, name=name)
    # x and the qkv weights arrive as fp8 value+residual pairs: the three
    # DoubleRow passes x8*w8 + x8*wr8 + xr8*w8 cost 6 rows/column where
    # bf16 costs 8. The weights are pre-scaled by 32 on the host so their
    # residuals clear the e4m3 subnormal floor (|w|~0.03 -> wr~0.002 would
    # quantize to ~2e-3 steps and cost 1.8% per element); the exact /32
    # rides the existing scale+bias drain ops.
    xT8 = nc.dram_tensor("xT8", [DK, NT], f8, kind="ExternalInput")
    xTr = nc.dram_tensor("xTr", [DK, NT], f8, kind="ExternalInput")
    wq8 = nc.dram_tensor("wq8", [DK, ch], f8, kind="ExternalInput")
    wqr = nc.dram_tensor("wqr", [DK, ch], f8, kind="ExternalInput")
    wk8 = nc.dram_tensor("wk8", [DK, ch], f8, kind="ExternalInput")
    wkr = nc.dram_tensor("wkr", [DK, ch], f8, kind="ExternalInput")
    wv8 = nc.dram_tensor("wv8", [DK, ch], f8, kind="ExternalInput")
    wvr = nc.dram_tensor("wvr", [DK, ch], f8, kind="ExternalInput")
    wvb = nc.dram_tensor("wvb_d", [1, ch], bf16, kind="ExternalInput")
    wo = nc.dram_tensor("wo", [ch, DO], bf16, kind="ExternalInput")
    bq = nc.dram_tensor("bq", [ch, 1], f32, kind="ExternalInput")
    bk = nc.dram_tensor("bk", [ch, 1], f32, kind="ExternalInput")
    out = nc.dram_tensor("out", [NT, DO], bf16, kind="ExternalOutput")

    with TileContext(nc) as tc:
        with tc.tile_pool(name="persist", bufs=1) as pp:
            x8 = pp.tile([128, KT, NT], f8)
            xr8 = pp.tile([128, KT, NT], f8)
            wq_sb = pp.tile([128, KT, ch], f8)
            wqr_sb = pp.tile([128, KT, ch], f8)
            wk_sb = pp.tile([128, KT, ch], f8)
            wkr_sb = pp.tile([128, KT, ch], f8)
            wv_sb = pp.tile([128, KT, ch], f8)
            wvr_sb = pp.tile([128, KT, ch], f8)
            wvb_sb = pp.tile([1, ch], bf16)
            wo_sb = pp.tile([128, CHT, DO], bf16)
            kT_sb = pp.tile([64, NH, NT], bf16)
            qT_sb = pp.tile([64, NH, NT], bf16)
            oT_sb = pp.tile([128, CHT, NT], bf16)
            v65 = pp.tile([128, TOKT, nh, 65], bf16)
            bqk = pp.tile([128, 2, CHT], f32)
            vbias_bc = pp.tile([128, nh, 64], bf16)
            zeros96 = pp.tile([128, 128], f8)
            ident = pp.tile([128, 128], bf16)
            warm = pp.tile([1, 256], f32r)
            warm_f = pp.tile([1, 256], f32)
            wact_i = pp.tile([1, 16], f32)
            wact_o = pp.tile([1, 16], f32)

            # warm-chain inputs first so the PE warmup starts immediately
            nc.vector.memset(warm_f[:], 1.0)
            nc.vector.tensor_copy(warm[:], warm_f[:])
            nc.vector.memset(zeros96[:], 0.0)
            nc.vector.memset(v65[:, :, :, 0:1], 1.0)
            nc.gpsimd.memset(wact_i[:], 0.0)
            masks.make_identity(nc, ident[:])
            # load the Exp table on ACT while DMAs run
            nc.scalar.activation(wact_o[:], wact_i[:], Exp, scale=1.0)

            with tc.tile_pool(name="stp", bufs=2, space="PSUM") as stp, \
                 tc.tile_pool(name="acp", bufs=2, space="PSUM") as acp, \
                 tc.tile_pool(name="pjp", bufs=2, space="PSUM") as pjp, \
                 tc.tile_pool(name="ptq", bufs=8) as ptq, \
                 tc.tile_pool(name="nrm", bufs=4) as nrm, \
                 tc.tile_pool(name="osg", bufs=4) as osg:

                # PE warmup: long accumulation chain of cheap 256-wide rows;
                # spans the x DMA so real matmuls start fully ramped.
                wps = pjp.tile([128, 512], f32, name="wps", tag="pj")
                NWARM = 24
                for i in range(NWARM):
                    nc.tensor.matmul(
                        wps[0:1, 0:256],
                        lhsT=warm[0:1, 0:1],
                        rhs=warm[0:1, :],
                        start=(i == 0),
                        stop=(i == NWARM - 1),
                    )

                # input DMAs: wk + x first (k streams during x); wk and x
                # split per-kt so the first K-proj round only waits on its
                # own slices; biases are only needed once the first psum
                # chains finish
                nc.sync.dma_start(
                    wk_sb[:], wk8[:, :].rearrange("(a p) c -> p a c", p=128)
                )
                nc.sync.dma_start(
                    wkr_sb[:], wkr[:, :].rearrange("(a p) c -> p a c", p=128)
                )
                for kt in range(KT):
                    nc.sync.dma_start(
                        x8[:, kt, :], xT8[128 * kt : 128 * (kt + 1), :]
                    )
                for kt in range(KT):
                    nc.sync.dma_start(
                        xr8[:, kt, :], xTr[128 * kt : 128 * (kt + 1), :]
                    )
                for i, bsrc in enumerate((bq, bk)):
                    for ct in range(CHT):
                        nc.sync.dma_start(
                            bqk[:, i, ct : ct + 1], bsrc[ct * 128 : (ct + 1) * 128, :]
                        )
                nc.sync.dma_start(
                    wq_sb[:], wq8[:, :].rearrange("(a p) c -> p a c", p=128)
                )
                nc.sync.dma_start(
                    wqr_sb[:], wqr[:, :].rearrange("(a p) c -> p a c", p=128)
                )
                nc.sync.dma_start(
                    wv_sb[:], wv8[:, :].rearrange("(a p) c -> p a c", p=128)
                )
                nc.sync.dma_start(
                    wvr_sb[:], wvr[:, :].rearrange("(a p) c -> p a c", p=128)
                )
                nc.sync.dma_start(wvb_sb[:], wvb[:, :])
                nc.sync.dma_start(
                    wo_sb[:], wo[:, :].rearrange("(a p) c -> p a c", p=128)
                )
                # v bias broadcast across partitions: the PV ones-column
                # gives o_c = PV_c + b_c*den, so adding b_c after the
                # normalize divide is exact -- no bias matmul needed
                nc.gpsimd.partition_broadcast(
                    vbias_bc[:], wvb_sb[:], channels=128
                )

                # k-projection: 8 chains (2 mt x 4 key-blocks of 512) spread
                # over all 8 psum banks, kt-outer so each x pair is consumed
                # as it lands.
                kf = [
                    stp.tile([128, 512], f32, name=f"kf{c}", tag="sta")
                    for c in range(2)
                ] + [
                    stp.tile([128, 512], f32, name=f"kf{c+2}", tag="stb")
                    for c in range(2)
                ] + [
                    acp.tile([128, 512], f32, name=f"kf{c+4}", tag="acc")
                    for c in range(2)
                ] + [
                    pjp.tile([128, 512], f32, name=f"kf{c+6}", tag="pj")
                    for c in range(2)
                ]
                chunks = [(kf[c][:], c % 2, c // 2) for c in range(8)]
                for phase, passes in enumerate(
                    (((wk_sb, x8), (wkr_sb, x8)), ((wk_sb, xr8),))
                ):
                    for p in range(KT // 2):
                        kp = slice(2 * p, 2 * p + 2)
                        for ps, mt, kb in chunks:
                            ms = slice(mt * 128, (mt + 1) * 128)
                            ts_ = slice(kb * 512, (kb + 1) * 512)
                            for i, (wt, xt_) in enumerate(passes):
                                nc.tensor.matmul(
                                    ps,
                                    lhsT=wt[:, kp, ms],
                                    rhs=xt_[:, kp, ts_],
                                    start=(phase == 0 and p == 0 and i == 0),
                                    stop=(phase == 1 and p == KT // 2 - 1),
                                    perf_mode=mybir.MatmulPerfMode.DoubleRow,
                                )

                def k_copy(c, scalar_eng=False):
                    ps, mt, kb = chunks[c]
                    for j in range(2):
                        if scalar_eng:
                            # ACT is idle during the front: Copy(in + bias)
                            # does the psum->sbuf bias-add there
                            nc.scalar.activation(
                                kT_sb[0:64, 2 * mt + j, kb * 512 : (kb + 1) * 512],
                                ps[64 * j : 64 * j + 64, :],
                                Ident,
                                bias=bqk[64 * j : 64 * j + 64, 1, mt : mt + 1],
                                scale=1.0 / 32.0,
                            )
                        else:
                            nc.vector.tensor_scalar(
                                kT_sb[0:64, 2 * mt + j, kb * 512 : (kb + 1) * 512],
                                ps[64 * j : 64 * j + 64, :],
                                1.0 / 32.0,
                                bqk[64 * j : 64 * j + 64, 1, mt : mt + 1],
                                mult,
                                mybir.AluOpType.add,
                            )

                def q_chunk(qt, mt, ps=None, scalar_eng=True):
                    if ps is None:
                        ps = pjp.tile([128, 512], f32, name="qps", tag="pj")
                    ms = slice(mt * 128, (mt + 1) * 128)
                    qs = slice(qt * QW, (qt + 1) * QW)
                    for p in range(KT // 2):
                        kp = slice(2 * p, 2 * p + 2)
                        for i, (wt, xt_) in enumerate(
                            ((wq_sb, x8), (wqr_sb, x8), (wq_sb, xr8))
                        ):
                            nc.tensor.matmul(
                                ps[:, 0:QW],
                                lhsT=wt[:, kp, ms],
                                rhs=xt_[:, kp, qs],
                                start=(p == 0 and i == 0),
                                stop=(p == KT // 2 - 1 and i == 2),
                                perf_mode=mybir.MatmulPerfMode.DoubleRow,
                            )
                    for j in range(2):
                        if scalar_eng:
                            nc.scalar.activation(
                                qT_sb[0:64, 2 * mt + j, qt * QW : (qt + 1) * QW],
                                ps[64 * j : 64 * j + 64, 0:QW],
                                Ident,
                                bias=bqk[64 * j : 64 * j + 64, 0, mt : mt + 1],
                                scale=1.0 / 32.0,
                            )
                        else:
                            nc.vector.tensor_scalar(
                                qT_sb[0:64, 2 * mt + j, qt * QW : (qt + 1) * QW],
                                ps[64 * j : 64 * j + 64, 0:QW],
                                1.0 / 32.0,
                                bqk[64 * j : 64 * j + 64, 0, mt : mt + 1],
                                mult,
                                mybir.AluOpType.add,
                            )

                def v_chunk(tt, ps=None):
                    if ps is None:
                        ps = pjp.tile([128, 512], f32, name="vps", tag="pj")
                    ts_ = slice(tt * 128, (tt + 1) * 128)
                    for p in range(KT // 2):
                        kp = slice(2 * p, 2 * p + 2)
                        for i, (xt_, wt) in enumerate(
                            ((x8, wv_sb), (x8, wvr_sb), (xr8, wv_sb))
                        ):
                            nc.tensor.matmul(
                                ps[:, 0:ch],
                                lhsT=xt_[:, kp, ts_],
                                rhs=wt[:, kp, :],
                                start=(p == 0 and i == 0),
                                stop=(p == KT // 2 - 1 and i == 2),
                                perf_mode=mybir.MatmulPerfMode.DoubleRow,
                            )
                    nc.vector.scalar_tensor_tensor(
                        v65[:, tt, :, 1:65],
                        ps[:, 0:ch],
                        1.0 / 32.0,
                        vbias_bc[:],
                        mult,
                        mybir.AluOpType.add,
                    )

                ostg = {}

                def o_piece(qt, tp, ntb, act_copy=None, st_ps=False, split_dma=False):
                    if st_ps:
                        ps = stp.tile([128, 512], f32, name="opst", tag="sta")
                    else:
                        ps = pjp.tile([128, 512], f32, name="ops", tag="pj")
                    tok0 = qt * QW + tp * 128
                    for ct in range(CHT):
                        nc.tensor.matmul(
                            ps[:],
                            lhsT=oT_sb[:, ct, tok0 : tok0 + 128],
                            rhs=wo_sb[:, ct, ntb * 512 : (ntb + 1) * 512],
                            start=(ct == 0),
                            stop=(ct == CHT - 1),
                        )
                    if ntb == 0:
                        ostg[tp] = osg.tile([128, DO], bf16, name="stg", tag="stg")
                    stg = ostg[tp]
                    # stage out through sbuf (psum can't DMA to dram); the
                    # two halves drain on different engines
                    if act_copy is None:
                        act_copy = stg_eng
                    dst = stg[:, ntb * 512 : (ntb + 1) * 512]
                    if act_copy == "act":
                        nc.scalar.activation(dst, ps[:], Ident)
                    elif act_copy == "pool":
                        nc.gpsimd.tensor_copy(dst, ps[:])
                    else:
                        nc.vector.tensor_copy(dst, ps[:])
                    if split_dma:
                        nc.sync.dma_start(
                            out[tok0 : tok0 + 128, ntb * 512 : (ntb + 1) * 512], dst
                        )
                    elif ntb == DO // 512 - 1:
                        nc.sync.dma_start(out[tok0 : tok0 + 128, :], stg[:])

                o_q = {}

                def t_piece(qt, qh):
                    # flip o (token-major bf16) to channel-major for the
                    # O-projection: two PE transposes + one strided DVE drain
                    tok0 = qt * QW + qh * 128
                    tp_ps = pjp.tile([128, 2, 128], bf16, name="tps", tag="pj")
                    for bp in range(2):
                        nc.tensor.transpose(
                            tp_ps[:, bp, :],
                            o_q[(qt, qh)][:, 2 * bp : 2 * bp + 2, :],
                            ident[:],
                        )
                    if t_eng == "act":
                        nc.scalar.activation(
                            oT_sb[:, :, tok0 : tok0 + 128], tp_ps[:], Ident
                        )
                    else:
                        nc.vector.tensor_copy(
                            oT_sb[:, :, tok0 : tok0 + 128], tp_ps[:]
                        )

                # front tail: kb0 copies on DVE free the first st slot for
                # the q0 chains; kb1 rides on the still-idle ACT (emitted
                # before anything q0-dependent so the counting semaphores
                # don't couple it); kb2/kb3 go last on DVE.
                # q0/v0/v1 land in st-slot banks (no bank sharing, no opens).
                k_copy(0)
                k_copy(1)
                k_copy(2, scalar_eng=True)
                k_copy(3, scalar_eng=True)
                q0ta = stp.tile([128, 512], f32, name="q0ta", tag="sta")
                q0tb = stp.tile([128, 512], f32, name="q0tb", tag="stb")
                q_chunk(0, 0, ps=q0ta[:], scalar_eng=True)
                q_chunk(0, 1, ps=q0tb[:], scalar_eng=True)
                # kb2/kb3 copies go on DVE before the v01 copies (whose psum
                # chains finish late) so body-0's schbf isn't queued behind
                # them
                for c in (4, 5, 6, 7):
                    k_copy(c)
                v01a = stp.tile([128, 512], f32, name="v01a", tag="sta")
                v01b = stp.tile([128, 512], f32, name="v01b", tag="stb")
                v_chunk(0, ps=v01a[:])
                v_chunk(1, ps=v01b[:])

                def emit(item):
                    if item[0] == "v":
                        v_chunk(item[1])
                    elif item[0] == "q":
                        q_chunk(item[1], item[2])
                    elif item[0] == "t":
                        t_piece(item[1], item[2])
                    else:
                        o_piece(item[1], item[2], item[3])

                def s_tile(qt, it, half):
                    # all operands at partition offset 0: offset-64 matmul
                    # operands combined with sub-bank psum outputs misbehave
                    # on hardware. heads 0,1 -> sta (ACT), heads 2,3 -> stb
                    # (DVE + Pool) so each exp engine recycles its own bank.
                    tag = "sta" if half == 0 else "stb"
                    st = stp.tile([128, 2, QW], f32, name=tag, tag=tag)
                    for j in range(2):
                        h = 2 * half + j
                        nc.tensor.matmul(
                            st[:, j, :],
                            lhsT=kT_sb[0:64, h, it * 128 : (it + 1) * 128],
                            rhs=qT_sb[0:64, h, qt * QW : (qt + 1) * QW],
                            start=True,
                            stop=True,
                        )
                    return st

                def allow(item, qt, it):
                    if qt == 0:
                        return True
                    kind = item[0]
                    if kind == "t":
                        return it in (2, 3, 4, 5)
                    if kind == "o":
                        return it >= 10
                    return it in (2, 3) or it >= 10

                if stage >= 13:
                    # qt0 stream order: v tiles must be emitted (program order)
                    # before the PV matmul of the same kt reads them
                    pending = deque()
                    pending += [("v", 2), ("v", 3), ("v", 4), ("v", 5)]
                    pending += [("q", 1, 0), ("q", 1, 1)]
                    pending += [("v", tt) for tt in range(6, TOKT)]

                    pvs = []
                    order = [(qt, it) for qt in range(QT) for it in range(TOKT)]
                    if stage in (13, 14):
                        order = order[:TOKT]
                    leftover = []

                    def do_normalize(qt):
                        # per-q-partition normalize: one reciprocal over the 4
                        # head denominators, then 4 per-partition multiplies
                        # emitting bf16 token-major o. The v-bias is baked
                        # into v65 during the v psum drain (sum p*(v+vb)/den
                        # = PV/den + vb), so this is a pure multiply; for the
                        # final qt the second half runs on the otherwise-idle
                        # ACT so the tail normalize halves.
                        acc = accs[qt]
                        for qh in range(2):
                            rcp = nrm.tile([128, nh, 1], f32, name="rcp", tag="rcp")
                            nc.vector.reciprocal(rcp[:], acc[qh][:, :, 0:1])
                            oq = nrm.tile([128, nh, 64], bf16, name="oq", tag="oq")
                            o_q[(qt, qh)] = oq
                            for h in range(nh):
                                if qt == QT - 1 and qh == 1:
                                    nc.scalar.activation(
                                        oq[:, h, :],
                                        acc[qh][:, h, 1:65],
                                        mybir.ActivationFunctionType.Copy,
                                        scale=rcp[:, h, 0:1],
                                    )
                                elif n_eng == "act":
                                    nc.scalar.activation(
                                        oq[:, h, :],
                                        acc[qh][:, h, 1:65],
                                        mybir.ActivationFunctionType.Copy,
                                        scale=rcp[:, h, 0:1],
                                    )
                                else:
                                    nc.vector.tensor_scalar_mul(
                                        oq[:, h, :],
                                        acc[qh][:, h, 1:65],
                                        rcp[:, h, 0:1],
                                    )
                        accs.pop(qt, None)
                        if qt + 2 < QT:
                            pending.extend([("q", qt + 2, 0), ("q", qt + 2, 1)])
                        if stage >= 40:
                            pending.extend([("t", qt, 0), ("t", qt, 1)])
                            for tp in range(QW // 128):
                                for ntb in range(DO // 512):
                                    pending.append(("o", qt, tp, ntb))

                    SCHB = 16196.5  # 127*128 + c  (c ~ -60, +0.5 trunc hedge)
                    SCHA = 23.083120654223414  # 128*log2(e)*scale

                    sts = {order[0]: (s_tile(*order[0], 0), s_tile(*order[0], 1))}
                    accs = {}
                    for n, (qt, it) in enumerate(order):
                        sta, stb = sts.pop((qt, it))
                        # heads 0,1: true exp on ACT; head 2: int16-bits
                        # schraudolph exp on DVE; head 3: same on gpsimd --
                        # separate pt tiles per engine so no writer pair ever
                        # serializes on a shared tile
                        pta = ptq.tile([128, 2, QW], bf16, name="pta", tag="pta")
                        ptd = ptq.tile([128, 2, QW], bf16, name="ptd", tag="ptd")
                        if stage >= 14:
                            nc.scalar.activation(pta[:], sta[:], Exp, scale=scale)
                            if ph:
                                # heads 2,3: exp via int16-bits-as-bf16 on DVE
                                # (gpsimd cannot read psum on this hw)
                                nc.vector.tensor_scalar(
                                    ptd[:].bitcast(i16),
                                    stb[:],
                                    SCHA,
                                    SCHB,
                                    mybir.AluOpType.mult,
                                    mybir.AluOpType.add,
                                )
                            else:
                                nc.scalar.activation(ptd[:], stb[:], Exp, scale=scale)
                        else:
                            nc.vector.tensor_copy(pta[:], sta[:])

                        def pt_slice(h, qh, pta=pta, ptd=ptd):
                            qs = slice(qh * 128, (qh + 1) * 128)
                            if h < 2:
                                return pta[:, h, qs]
                            return ptd[:, h - 2, qs]
                        # for qt>0 the accumulator banks are freed by the
                        # previous qt's normalize reads: defer the opens to
                        # body 2 and the first PV groups to body 3 so the wait
                        # never head-of-line-blocks the in-order PE stream
                        open_at = 0 if qt == 0 else oat
                        pv_at = 0 if qt == 0 else pat
                        if it == open_at and stage >= 17:
                            accs[qt] = [
                                acp.tile([128, nh, 65], f32, name="acc", tag="acc")
                                for _ in range(2)
                            ]
                            # open each accumulator bank with one 260-wide zero
                            # matmul: psum start resets the whole 2KB zero
                            # region, so the chains sharing a bank must not
                            # each start
                            for a in accs[qt]:
                                nc.tensor.matmul(
                                    a[:],
                                    lhsT=zeros96[:],
                                    rhs=x8[:, 0, 0:260],
                                    start=True,
                                    stop=False,
                                    skip_group_check=True,
                                )

                        if not s_after and n + 1 < len(order):
                            sts[order[n + 1]] = (
                                s_tile(*order[n + 1], 0), s_tile(*order[n + 1], 1)
                            )
                        # leftover PV group + normalize of the previous qt
                        # ride body 0 here, so the body-15 flush never waits
                        # on the freshest pt and PE never sits through the
                        # normalize
                        if leftover:
                            for lqt, lit, lpsl in leftover:
                                lacc = accs[lqt]
                                for qh in range(2):
                                    for h in range(nh):
                                        nc.tensor.matmul(
                                            lacc[qh][:, h, :],
                                            lhsT=lpsl(h, qh),
                                            rhs=v65[:, lit, h, :],
                                            start=False,
                                            stop=(lit == TOKT - 1),
                                            skip_group_check=True,
                                        )
                            lq = leftover[0][0]
                            leftover.clear()
                            do_normalize(lq)
                        acc = accs.get(qt)
                        if stage >= 17:
                            pvs.append((qt, it, pt_slice))
                            if it >= pv_at:
                                # catch up the deferred backlog two groups per
                                # body, then stream one per body so the final
                                # body never bursts and starves ACT/Pool
                                if it == TOKT - 1:
                                    # flush all but the freshest group; it and
                                    # the normalize ride body 0 of the next qt
                                    ndrain = len(pvs) - (1 if qt + 1 < QT else 0)
                                elif catchup and len(pvs) > 1:
                                    ndrain = 2
                                else:
                                    ndrain = 1
                                for pqt, pit, psl in pvs[:ndrain]:
                                    for qh in range(2):
                                        for h in range(nh):
                                            nc.tensor.matmul(
                                                acc[qh][:, h, :],
                                                lhsT=psl(h, qh),
                                                rhs=v65[:, pit, h, :],
                                                start=False,
                                                stop=(pit == TOKT - 1),
                                                skip_group_check=True,
                                            )
                                del pvs[:ndrain]
                        if s_after and n + 1 < len(order):
                            sts[order[n + 1]] = (
                                s_tile(*order[n + 1], 0), s_tile(*order[n + 1], 1)
                            )
                        if it == TOKT - 1 and stage >= 30:
                            if qt + 1 < QT:
                                leftover.extend(pvs)
                                del pvs[:]
                            else:
                                do_normalize(qt)
                        if stage >= 16:
                            # q/v chunks are normalize-independent and may fill
                            # the early (PE-idle) bodies; transposes wait on
                            # the previous qt's normalize so they ride bodies
                            # 2-5, o pieces (behind the transpose drain) only
                            # stream from body 10
                            if qt == 0:
                                k = 2 if it <= 3 else 1
                            else:
                                k = 1 if (it in (2, 3, 4, 5) or it >= 10) else 0
                            for _ in range(k):
                                if pending and allow(pending[0], qt, it):
                                    emit(pending.popleft())
                    ndrain2 = 0
                    while pending:
                        # post-loop drain: every engine is idle once the exps
                        # are done; cycle the stg copies across ACT/DVE/Pool
                        # and DMA each 512-half as soon as it lands
                        item = pending.popleft()
                        if item[0] == "o":
                            # the st banks are idle after the last exp: run
                            # half the drain chains there so four pieces are
                            # in flight instead of two
                            o_piece(
                                item[1], item[2], item[3],
                                act_copy=("act", "dve")[ndrain2 % 2],
                                st_ps=(ndrain2 % 2 == 1),
                                split_dma=True,
                            )
                            ndrain2 += 1
                        else:
                            emit(item)
    nc.finalize()
    return nc


def make_in_maps(x, W_qkv, b_qkv, W_o):
    """Shard full inputs into per-core input maps (core c: batch c//4, group c%4)."""
    import ml_dtypes

    BF = ml_dtypes.bfloat16
    F8 = ml_dtypes.float8_e4m3
    x = np.asarray(x, dtype=np.float32)
    W_qkv = np.asarray(W_qkv, dtype=np.float32)
    b_qkv = np.asarray(b_qkv, dtype=np.float32)
    W_o = np.asarray(W_o, dtype=np.float32)
    in_maps = []
    for c in range(NCORES):
        b, g = divmod(c, GROUPS)
        cols = slice(CH * g, CH * (g + 1))

        xT = np.ascontiguousarray(x[b].T)
        x8 = xT.astype(F8)
        xr = (xT - x8.astype(np.float32)).astype(F8)

        def w8r(i):
            # weights pre-scaled x32 so the fp8 residual clears the e4m3
            # subnormal floor; the kernel divides by 32 in the psum drains
            w = W_qkv[:, i * D : (i + 1) * D][:, cols] * 32.0
            w8 = w.astype(F8)
            wr = (w - w8.astype(np.float32)).astype(F8)
            return np.ascontiguousarray(w8), np.ascontiguousarray(wr)

        q8, qr = w8r(0)
        k8, kr = w8r(1)
        v8, vr = w8r(2)
        m = {
            "xT8": x8,
            "xTr": xr,
            "wq8": q8,
            "wqr": qr,
            "wk8": k8,
            "wkr": kr,
            "wv8": v8,
            "wvr": vr,
            "wvb_d": np.ascontiguousarray(
                b_qkv[2 * D : 3 * D][cols][None, :]
            ).astype(BF),
            "wo": np.ascontiguousarray(W_o[cols, :]).astype(BF),
            "bq": np.ascontiguousarray(b_qkv[0 * D : 1 * D][cols][:, None]),
            "bk": np.ascontiguousarray(b_qkv[1 * D : 2 * D][cols][:, None]),
        }
        in_maps.append(m)
    return in_maps


_NC = None


def get_nc():
    global _NC
    if _NC is None:
        _NC = build_nc()
    return _NC


def kernel(x, W_qkv, b_qkv, W_o, b_o):
    from concourse import bass_utils

    b_o = np.asarray(b_o, dtype=np.float32)
    in_maps = make_in_maps(x, W_qkv, b_qkv, W_o)
    res = bass_utils.run_bass_kernel_spmd(get_nc(), in_maps, core_ids=list(range(NCORES)))
    out = np.empty((B, N, D), dtype=np.float32)
    for b in range(B):
        acc = np.asarray(res.results[4 * b]["out"], dtype=np.float32)
        for g in range(1, GROUPS):
            acc += np.asarray(res.results[4 * b + g]["out"], dtype=np.float32)
        out[b] = acc + b_o
    return out
